# revision 19
# baseline (speedup 1.0000x reference)
"""ContrastiveLoss kernel for 8x Trainium2 NeuronCores.

Math: loss = mean_i ||o2_i - o1_i||^2  +  mean_i relu(MARGIN - d_i)
where d_i is the rn_i-th smallest entry of row i of the [N,N] distance
matrix dist(i,j) = ||o2_j - o1_i|| (with a self-match index rejection).

Every candidate d_i is >= min_j dist(i,j). So whenever we can PROVE
min_j dist(i,j) >= MARGIN for all i, the second term is exactly 0 and
loss == mean(pos). The proof needs an upper bound on max_j <o1_i,o2_j>.
Split coordinates: <a,b> = <a[:KH],b[:KH]> + <a[KH:],b[KH:]> with
KH = 512. The kernel computes the HALF Gram o1[:, :KH] @ o2[:, :KH]^T
on the PE arrays (fp8 DoubleRow; half the MACs of the full Gram) and
reduces each row to an upper bound on max_j <a[:KH],b[KH:]>; the tail
term is bounded by Cauchy-Schwarz |a[KH:]|*max_j |b[KH:]| from exact
host-side row norms. With O(1) gaussian data the resulting rigorous
bound on min dist^2 clears MARGIN^2 = 4 by ~280 (verified numerically),
covering worst-case fp8 rounding of the matmul inputs. If the check
ever failed, an exact host fallback reproduces the reference.

The positive term: the device computes per-row dots <o1_i, o2_i> from
bf16 copies (worst-case error ~0.5% of pos, vs the 2e-2 harness
tolerance) and the host assembles pos_i = a2_i + b2_i - 2 dot_i with
exact fp64 row norms.

Sharding: rows of output1 split across the 8 cores (1024 rows each);
output2[:, :KH]^T replicated per core. Each core computes its
[1024, 8192] half-Gram block, per-1024-column row maxima (DVE exact
max / ACT log-sum-exp, alternating so the PSUM drain splits across
both engines), and its rows' diagonal dots. Scalar assembly happens
on host (a few KB per core of output).
"""

import numpy as np
import ml_dtypes

N = 8192
D = 1024
KH = 384           # Gram contraction (first KH coordinates)
NCORES = 8
MPC = N // NCORES  # rows per core = 1024
P = 128
MT = MPC // P      # 8 m-tiles per core
KT = KH // P       # 3 k-tiles
USE_DR = True      # 1 DoubleRow pair + 1 Normal tile: the mode alternation
                   # lets each LDWEIGHTS overlap the other mode's matmuls
                   # (measured: hybrid 65us vs all-DR-k512 96us scaled,
                   # all-Normal 127us)
KP = KT // 2 if USE_DR else 0   # DoubleRow k-pairs
KS = KT - 2 * KP                # Normal-mode single k-tiles
NFREE = 512        # matmul free dim (one PSUM bank)
PAIRW = 2 * NFREE  # two banks reduced per DVE/ACT instruction
NPAIR = N // PAIRW  # 8 pair-column blocks per m-tile
GCOLS = MT * NPAIR  # 64 reduce columns
MARGIN = 2.0
QUANT = 30
T_LSE = 0.125  # log-sum-exp temperature: lse/T >= row max of the Gram block
NCHUNK = 4     # rhs loads split into column chunks so PE starts early
SWI = False    # DoubleRowSwInterleave (measured slower on HW; keep off)

_PROG = None
LAST_RESULTS = None  # BassKernelResults of the most recent run (for test harness)
LAST_BOUND = None    # min certified distance^2 bound of the most recent run
LAST_FASTPATH = None


def _build_program(reps=1, mode="full", act_fn="exp", hg=2, sing_q="sync",
                   drain="mix", dgran=2, drain_accum=True, drain_static=False,
                   pos_dev=False, esc_pool="escp"):
    """mode: 'full' (normal), 'dma' (loads only), 'mm' (matmuls + trivial
    drain), 'compute' (loads hoisted out of the timing loop)."""
    import contextlib

    import concourse.bacc as bacc
    import concourse.mybir as mybir
    import concourse.tile as tile

    nc = bacc.Bacc(None, target_bir_lowering=False, debug=False)
    f32 = mybir.dt.float32
    bf16 = mybir.dt.bfloat16
    fp8 = mybir.dt.float8e4
    X = mybir.AxisListType.X
    Alu = mybir.AluOpType
    DR = mybir.MatmulPerfMode.DoubleRow

    f16 = mybir.dt.float16
    lhsT_d = nc.dram_tensor("lhsT", [KH, MPC], fp8, kind="ExternalInput")
    rhsT_d = nc.dram_tensor("rhsT", [KH, N], fp8, kind="ExternalInput")
    # d = o1 - o2 rows (fp16, host-subtracted); pos_i = sum_k d_ik^2
    db_d = (
        nc.dram_tensor("db", [MPC, D], f16, kind="ExternalInput")
        if pos_dev
        else None
    )
    # Row-max info per 1024-wide j block, col = m*NPAIR + h. The
    # blocks alternate between the two engines that can read PSUM:
    #   (m+h) even -> DVE exact max into gmax; (m+h) odd -> ACT
    #   sum_j exp(T_LSE*g) into sexp (log-sum-exp row-max bound).
    # Unwritten cols stay 0 (outputs are pre-zeroed): 0 only loosens the
    # host-side max, and adds 0 to the exp sum.
    gmax_d = nc.dram_tensor("gmax", [P, GCOLS], f32, kind="ExternalOutput")
    sexp_d = nc.dram_tensor("sexp", [P, GCOLS], f32, kind="ExternalOutput")
    # dot[p, m] = sum_k d[r,k]^2 = pos loss for row r = m*128+p
    dot_d = (
        nc.dram_tensor("dot", [P, MT], f32, kind="ExternalOutput")
        if pos_dev
        else None
    )

    with tile.TileContext(nc) as tc:
        with (
            tc.tile_pool(name="persist", bufs=1) as persist,
            tc.tile_pool(name="lhs", bufs=4) as lhsp,
            tc.tile_pool(name="rhs", bufs=4) as rhsp,
            tc.tile_pool(name="posin", bufs=16) as posin,
            tc.tile_pool(name="scratch", bufs=2) as scratch,
            tc.tile_pool(name="escp", bufs=4) as escp,
            tc.tile_pool(
                name="psum",
                bufs=(8 if not drain_static else 6) // (hg * dgran),
                space="PSUM",
            ) as psum,
            tc.tile_pool(name="psumst", bufs=1, space="PSUM") as psumst,
        ):
            gmax_sb = persist.tile([P, GCOLS], f32)
            sexp_sb = persist.tile([P, GCOLS], f32)
            dot_sb = persist.tile([P, MT], f32) if pos_dev else None
            nc.vector.memset(gmax_sb[:], 0.0)
            nc.scalar.memzero(sexp_sb[:])

            def load_inputs():
                # lhsT first (small, needed for every group), then rhs in
                # column chunks so the first groups' operands land early.
                # All weight loads ride the SP HWDGE queue; the bf16 dot
                # inputs ride the Activation HWDGE queue in parallel.
                lhs_pair = []
                lhs_sing = []
                rhs_pair = []
                rhs_sing = []
                for t in range(KP):
                    tl = lhsp.tile([P, 2, MPC], fp8, tag="lh", name=f"lhsT_{t}")
                    src = lhsT_d[2 * t * P : (2 * t + 2) * P, :]
                    nc.sync.dma_start(tl[:], src.rearrange("(r p) j -> p r j", p=P))
                    lhs_pair.append(tl)
                    rhs_pair.append(
                        rhsp.tile([P, 2, N], fp8, tag="rch", name=f"rch_{t}")
                    )
                for s in range(KS):
                    row0 = (2 * KP + s) * P
                    tl = lhsp.tile([P, MPC], fp8, tag="lhs", name=f"lhsS_{s}")
                    nc.sync.dma_start(tl[:], lhsT_d[row0 : row0 + P, :])
                    lhs_sing.append(tl)
                    rhs_sing.append(
                        rhsp.tile([P, N], fp8, tag="rsg", name=f"rsg_{s}")
                    )
                CW = N // NCHUNK
                for c in range(NCHUNK):
                    for t in range(KP):
                        rsrc = rhsT_d[2 * t * P : (2 * t + 2) * P, c * CW : (c + 1) * CW]
                        nc.sync.dma_start(
                            rhs_pair[t][:, :, c * CW : (c + 1) * CW],
                            rsrc.rearrange("(r p) j -> p r j", p=P),
                        )
                    for s in range(KS):
                        # single-tile rhs can ride either queue; the
                        # Activation queue otherwise only carries the 2MB
                        # of pos inputs
                        row0 = (2 * KP + s) * P
                        (nc.scalar if sing_q == "scalar" else nc.sync).dma_start(
                            rhs_sing[s][:, c * CW : (c + 1) * CW],
                            rhsT_d[row0 : row0 + P, c * CW : (c + 1) * CW],
                        )
                return (lhs_pair, lhs_sing), (rhs_pair, rhs_sing)

            HG = hg  # psum pair-tiles per stationary-reuse group

            def group_matmuls(m, hg, pts, rhs, lhs):
                # stationary outermost: each DoubleRow LDWEIGHTS is
                # amortized over 2*HG matmuls (DoubleRow disables FWL, so
                # stationary switches are expensive); the leftover single
                # k-tile runs Normal mode (FWL keeps its LDWEIGHTS cheap)
                hs = [hg * HG + i for i in range(HG)]
                rhs_pair, rhs_sing = rhs
                lhs_pair, lhs_sing = lhs
                for t in range(KP):
                    for i, h in enumerate(hs):
                        for half in range(2):
                            nc.tensor.matmul(
                                pts[i][:, half * NFREE : (half + 1) * NFREE],
                                lhs_pair[t][:, :, m * P : (m + 1) * P],
                                rhs_pair[t][
                                    :,
                                    :,
                                    (h * 2 + half) * NFREE : (h * 2 + half + 1) * NFREE,
                                ],
                                start=(t == 0),
                                stop=(t == KP - 1 and KS == 0),
                                perf_mode=DR,
                            )
                for s in range(KS):
                    for i, h in enumerate(hs):
                        for half in range(2):
                            nc.tensor.matmul(
                                pts[i][:, half * NFREE : (half + 1) * NFREE],
                                lhs_sing[s][:, m * P : (m + 1) * P],
                                rhs_sing[s][
                                    :,
                                    (h * 2 + half) * NFREE : (h * 2 + half + 1) * NFREE,
                                ],
                                start=(KP == 0 and s == 0),
                                stop=(s == KS - 1),
                            )

            st_tiles = (
                [psumst.tile([P, PAIRW], f32, name="st_drain")] if drain_static else []
            )
            if drain_static:
                nc.vector.memset(st_tiles[0][:], 0.25)

            def drain_one(m, src, col, width, use_dve):
                if use_dve:
                    nc.vector.tensor_reduce(
                        gmax_sb[:, col : col + 1], src, axis=X, op=Alu.max
                    )
                else:
                    # exp into a throwaway bf16 SBUF tile (avoids a
                    # same-bank PSUM read+write, and bf16 out enables
                    # the ScalarE 2x mode); only the accumulated fp32
                    # row sum is kept
                    pool = scratch if esc_pool == "scratch" else escp
                    esc = pool.tile(
                        [P, width], bf16, tag="esc", name=f"esc_{m}_{col}"
                    )
                    nc.scalar.activation(
                        esc[:],
                        src,
                        (
                            mybir.ActivationFunctionType.Exp
                            if act_fn == "exp"
                            else mybir.ActivationFunctionType.Copy
                        ),
                        bias=0.0,
                        scale=T_LSE,
                        accum_out=(
                            sexp_sb[:, col : col + 1] if drain_accum else None
                        ),
                    )

            def do_group(m, hg, rhs, lhs):
                hs = [hg * HG + i for i in range(HG)]
                if dgran == 1:
                    pts = [
                        psum.tile([P, PAIRW], f32, tag="acc", name=f"acc_{m}_{h}")
                        for h in hs
                    ]
                else:
                    quad = psum.tile(
                        [P, HG * PAIRW], f32, tag="acc", name=f"acc_{m}_{hg}"
                    )
                    pts = [quad[:, i * PAIRW : (i + 1) * PAIRW] for i in range(HG)]
                group_matmuls(m, hg, pts, rhs, lhs)
                if drain_static:
                    # timing bisect: tiny DVE drain recycles the group banks;
                    # the real-size drain reads a static psum pair instead
                    # (breaks the RAW dep on the matmuls)
                    for i, h in enumerate(hs):
                        col = m * NPAIR + h
                        nc.vector.tensor_reduce(
                            gmax_sb[:, col : col + 1], pts[i][:, 0:2], axis=X,
                            op=Alu.max,
                        )
                        use_dve = (
                            ((m + h) % 2 == 0) if drain == "mix" else (drain == "dve")
                        )
                        if not use_dve:
                            drain_one(m, st_tiles[0][:], col, PAIRW, False)
                    return
                if dgran == 1:
                    for i, h in enumerate(hs):
                        col = m * NPAIR + h
                        # one DVE (exact max) + one ACT (log-sum-exp) drain per
                        # group, so each engine owes ~1us per ~2us of group PE
                        # time -- both always keep up
                        use_dve = (
                            ((m + h) % 2 == 0) if drain == "mix" else (drain == "dve")
                        )
                        drain_one(m, pts[i][:], col, PAIRW, use_dve)
                else:
                    col = m * NPAIR + hs[0]
                    use_dve = (
                        ((m + hg) % 2 == 0) if drain == "mix" else (drain == "dve")
                    )
                    drain_one(m, quad[:], col, HG * PAIRW, use_dve)

            def load_pos(m):
                t = posin.tile([P, D], f16, tag="pin", name=f"pd_{m}")
                nc.scalar.dma_start(t[:], db_d[m * P : (m + 1) * P, :])
                return (t,)

            def do_dot(m, t):
                junk = scratch.tile([P, D], f16, tag="junk", name=f"junk_{m}")
                nc.vector.scalar_tensor_tensor(
                    out=junk[:],
                    in0=t[:],
                    scalar=1.0,
                    in1=t[:],
                    op0=Alu.bypass,
                    op1=Alu.mult,
                    accum_out=dot_sb[:, m : m + 1],
                )

            def do_all_blocks(rhs, lhs, pos_tiles=None):
                # dots are interleaved after each m-tile's groups: they sit
                # BEHIND the PSUM drains in the DVE FIFO (a dot waiting on
                # its DMA at the head of the queue would stall every drain
                # behind it, and with them the PE's bank recycling)
                for m in range(MT):
                    for hg in range(NPAIR // HG):
                        do_group(m, hg, rhs, lhs)
                    if pos_tiles is not None:
                        do_dot(m, *pos_tiles[m])

            body_ctx = (
                tc.For_i(0, reps, 1) if reps > 1 else contextlib.nullcontext()
            )
            if mode == "compute":
                # hoist every load out of the timing loop
                lhs, rhs = load_inputs()
                pos_tiles = [load_pos(m) for m in range(MT)]
                with body_ctx:
                    do_all_blocks(rhs, lhs, pos_tiles)
                # outputs once, after the loop: an in-loop output DMA on the
                # sync queue would head-block the NEXT iteration's input
                # loads behind this iteration's last drain
                if pos_dev:
                    nc.sync.dma_start(dot_d[:], dot_sb[:])
                nc.sync.dma_start(gmax_d[:], gmax_sb[:])
                nc.sync.dma_start(sexp_d[:], sexp_sb[:])
            elif mode == "mm":
                # matmuls only: every psum tile still drained, but by a
                # single cheap DVE reduce into one throwaway column
                lhs, rhs = load_inputs()
                junk = persist.tile([P, 1], f32, name="junk")
                with body_ctx:
                    for m in range(MT):
                        for hg in range(NPAIR // HG):
                            hs = [hg * HG + i for i in range(HG)]
                            pts = [
                                psum.tile([P, PAIRW], f32, tag="acc", name=f"acc_{m}_{h}")
                                for h in hs
                            ]
                            group_matmuls(m, hg, pts, rhs, lhs)
                            for i in range(HG):
                                nc.vector.tensor_reduce(
                                    junk[:], pts[i][:, 0:2], axis=X, op=Alu.max
                                )
                nc.sync.dma_start(gmax_d[:, 0:1], junk[:])
            elif mode == "dma":
                with body_ctx:
                    load_inputs()
                    for m in range(MT):
                        load_pos(m)
            else:
                with body_ctx:
                    lhs, rhs = load_inputs()
                    # all pos-load triggers upfront (posin is not recycled
                    # within an iteration, so none of these waits)
                    pos_tiles = (
                        [load_pos(m) for m in range(MT)] if pos_dev else None
                    )
                    do_all_blocks(rhs, lhs, pos_tiles)
                # outputs once, after the loop: an in-loop output DMA on the
                # sync queue would head-block the NEXT iteration's input
                # loads behind this iteration's last drain
                if pos_dev:
                    nc.sync.dma_start(dot_d[:], dot_sb[:])
                nc.sync.dma_start(gmax_d[:], gmax_sb[:])
                nc.sync.dma_start(sexp_d[:], sexp_sb[:])

    nc.compile()
    return nc


def _get_program():
    global _PROG
    if _PROG is None:
        _PROG = _build_program()
    return _PROG


def _exact_fallback(o1, o2, rn):
    """Faithful numpy mirror of the reference (fp32 ops, lax.top_k ties)."""
    o1 = o1.astype(np.float32)
    o2 = o2.astype(np.float32)
    pos = ((o2 - o1) ** 2).sum(axis=1, dtype=np.float32)
    a2 = (o1**2).sum(axis=1, dtype=np.float32)
    b2 = (o2**2).sum(axis=1, dtype=np.float32)
    neg = np.empty(N, np.float32)
    rows = np.arange(N)
    blk = 512
    for s in range(0, N, blk):
        e = min(s + blk, N)
        gram = o1[s:e] @ o2.T
        sq = a2[s:e, None] + b2[None, :] - 2.0 * gram
        dist = np.sqrt(np.maximum(sq, 0.0)).astype(np.float32)
        for r in range(s, e):
            drow = dist[r - s]
            # 30 smallest, ties broken by lower index (lax.top_k semantics)
            part = np.argpartition(drow, QUANT - 1)[: QUANT + 32]
            order = part[np.lexsort((part, drow[part]))]
            # lexsort of the partition prefix is only safe if the boundary
            # value isn't tied beyond the prefix; redo exactly if in doubt
            v_k = drow[order[QUANT - 1]]
            if (drow == v_k).sum() > (drow[order[:QUANT]] == v_k).sum():
                order = np.lexsort((rows, drow))
            idx = order[:QUANT]
            vals = drow[idx]
            r_sel = int(rn[r]) % QUANT
            if idx[r_sel] == r:
                r_sel = (r_sel + 1) % QUANT
            neg[r] = vals[r_sel]
    neg_loss = np.maximum(np.float32(MARGIN) - neg, np.float32(0.0))
    return np.float32(
        np.mean(pos, dtype=np.float64) + np.mean(neg_loss, dtype=np.float64)
    )


def make_in_maps(output1, output2, pos_dev=False):
    o1 = np.ascontiguousarray(np.asarray(output1, dtype=np.float32))
    o2 = np.ascontiguousarray(np.asarray(output2, dtype=np.float32))
    fp8 = ml_dtypes.float8_e4m3  # TRN E4M3: max normal +-240
    # inputs are O(1); clip defensively so adversarial values can't hit inf/NaN
    o1c = np.clip(o1, -224.0, 224.0)
    o2c = np.clip(o2, -224.0, 224.0)
    o1T_8 = np.ascontiguousarray(o1c[:, :KH].T.astype(fp8))      # [KH, N]
    o2T_8 = np.ascontiguousarray(o2c[:, :KH].T.astype(fp8))      # [KH, N]
    # fp32 subtract matches the reference's own rounding; fp16 storage adds
    # at most 2^-11 relative per element (overflow -> inf -> host fallback)
    d16 = (o1 - o2).astype(np.float16) if pos_dev else None

    in_maps = []
    for c in range(NCORES):
        sl = slice(c * MPC, (c + 1) * MPC)
        m = {
            "lhsT": np.ascontiguousarray(o1T_8[:, sl]),
            "rhsT": o2T_8,
        }
        if pos_dev:
            m["db"] = np.ascontiguousarray(d16[sl])
        in_maps.append(m)
    return in_maps


def kernel(output1, output2, rn):
    global LAST_RESULTS
    o1 = np.ascontiguousarray(np.asarray(output1, dtype=np.float32))
    o2 = np.ascontiguousarray(np.asarray(output2, dtype=np.float32))
    rn_np = np.asarray(rn)

    in_maps = make_in_maps(o1, o2)

    from concourse.bass_utils import run_bass_kernel_spmd

    nc = _get_program()
    res = run_bass_kernel_spmd(nc, in_maps, list(range(NCORES)))
    LAST_RESULTS = res

    sexp_rows = np.empty(N, np.float64)
    gmax_rows = np.empty(N, np.float64)
    for c in range(NCORES):
        sxc = np.asarray(res.results[c]["sexp"], dtype=np.float64)    # [P, GCOLS]
        gmc = np.asarray(res.results[c]["gmax"], dtype=np.float64)    # [P, GCOLS]
        for m in range(MT):
            base = c * MPC + m * P
            cols = slice(m * NPAIR, (m + 1) * NPAIR)
            sexp_rows[base : base + P] = sxc[:, cols].sum(axis=1)
            gmax_rows[base : base + P] = gmc[:, cols].max(axis=1)
    # pos term on host, exact fp64 (O(N*D): trivial vs the O(N^2 KH) Gram
    # certificate the device computes)
    d64 = o1.astype(np.float64) - o2.astype(np.float64)
    pos_rows = (d64 * d64).sum(axis=1)

    # Exact fp64 row norms (host, O(N*D) -- cheap vs the O(N^2 D) Gram).
    o1_64 = o1.astype(np.float64)
    o2_64 = o2.astype(np.float64)
    a2 = (o1_64**2).sum(axis=1)
    b2 = (o2_64**2).sum(axis=1)

    # Rigorous zero-check for the margin term, on the KH-coordinate split:
    #   <a,b> <= <a[:KH],b[:KH]> + |a[KH:]| * max_j |b[KH:]|
    nA2 = (o1_64[:, :KH] ** 2).sum(axis=1)
    nB2 = (o2_64[:, :KH] ** 2).sum(axis=1)
    nA = np.sqrt(nA2)
    nBmax = float(np.sqrt(nB2.max()))
    tA = np.sqrt(np.maximum(a2 - nA2, 0.0))
    uB = float(np.sqrt(np.maximum(b2 - nB2, 0.0).max()))
    amax = float(np.sqrt(a2.max()))
    bmax = float(np.sqrt(b2.max()))
    # log-sum-exp upper bound on row max: lse/T >= max_j g_fp8; the +1.0
    # covers the ACT Exp LUT relative error and fp32 accumulation of the
    # sum. Unwritten cols contributed 0 to the sum and 0 to the max (a 0
    # only loosens the upper bound). Final bound = max of the two halves.
    lse_ub = np.log(np.maximum(sexp_rows, 1e-30)) / T_LSE + 1.0
    gmax_ub = np.maximum(gmax_rows, lse_ub)
    # fp8 e4m3 round-to-nearest rel err 2^-4 per input element (+ clip is a
    # no-op for in-range data): |g_fp8 - g| <= (2*2^-4 + 2^-8)*|a_h||b_h|,
    # plus fp32 accumulation noise
    slack_g = 0.1330 * nA * nBmax + 0.1
    # clip shifts elements > 224 by at most their value; if any were clipped,
    # take the fallback (cannot certify)
    clipped = (np.abs(o1) > 224.0).any() or (np.abs(o2) > 224.0).any()
    # reference computes sq in fp32 from fp32 inputs; cover its roundoff too
    eps_ref = 1e-3 * amax * bmax + 1e-2
    bound = a2 + b2.min() - 2.0 * (gmax_ub + slack_g + tA * uB)
    global LAST_BOUND, LAST_FASTPATH
    LAST_BOUND = float(bound.min())
    # pos is computed on host in fp64 -- exact up to the fp32-vs-fp64
    # difference of the reference itself (covered by eps_ref's margin).
    pos_mean = float(np.mean(pos_rows))
    LAST_FASTPATH = (
        not clipped
        and bool(np.isfinite(bound).all())
        and LAST_BOUND >= MARGIN * MARGIN + eps_ref
    )
    if LAST_FASTPATH:
        return np.float32(pos_mean)
    return _exact_fallback(o1, o2, rn_np)

